# revision 8
# baseline (speedup 1.0000x reference)
"""NoisyTopkRouter kernel for Trainium2 (8 NeuronCores, data-parallel on tokens).

Math per token t (row of mh):
    logits  = mh @ W_route.T            (+ b_route, asserted zero)
    nlogits = mh @ W_noise.T            (+ b_noise, asserted zero)
    noisy   = logits + eps * softplus(nlogits)
    top-8 of noisy over the 64 experts  -> indices (descending), mask
    router  = softmax(where(mask, noisy, -inf))

Device strategy per core (2048 tokens, D=4096):
  - Both gemms fused into one: Wall = [W_route; W_noise] -> [128, 4096].
  - Contraction over d needs d on SBUF partitions for both matmul operands, so
    mh is loaded with a block-permuted DMA (32-wide sub-blocks) and fixed up
    with the DVE 32x32 stream transpose, giving mhT chunks [128 d, T tokens].
  - fp32 matmuls (exact) accumulate [128 tok, 128 e] PSUM tiles over 32
    d-chunks; tokens stay on partitions for the epilogue. Each 128-token
    block accumulates in its own PSUM bank (HW zero-region constraint).
  - softplus = Ln(1 + Exp(x)) on ACT (|x| small, no overflow), top-8 via the
    DVE MAX8 / FIND_INDEX_8 hardware, softmax via exp(noisy - (m + ln(sum))).
  - Elementwise epilogue spread over GPSIMD (else idle) to keep DVE free for
    the stream transposes.
"""

import numpy as np

B, S, D, E, K = 4, 4096, 4096, 64, 8
E2 = 2 * E  # stacked route+noise experts
N_CORES = 8
T_FULL = B * S
T_CORE = T_FULL // N_CORES


def build_nc(t_core=T_CORE, n_cores=N_CORES, debug=False, gp_elemwise=False):
    import concourse.bacc as bacc
    import concourse.mybir as mybir
    from concourse.tile import TileContext

    f32 = mybir.dt.float32
    u32 = mybir.dt.uint32
    Exp = mybir.ActivationFunctionType.Exp
    Ln = mybir.ActivationFunctionType.Ln
    X = mybir.AxisListType.X
    add = mybir.AluOpType.add
    is_ge = mybir.AluOpType.is_ge

    T = t_core
    TG = min(512, T)  # tokens per group
    G = T // TG  # groups
    TAU = TG // 128  # 128-token tiles per group
    BB = TG // 32  # 32-token blocks per group
    C = D // 128  # d-chunks (32)
    assert T % TG == 0 and TG % 128 == 0

    nc = bacc.Bacc("TRN2", target_bir_lowering=False, debug=debug, num_devices=n_cores)

    mh = nc.dram_tensor("mh", [T, D], f32, kind="ExternalInput")
    w_route = nc.dram_tensor("w_route", [E, D], f32, kind="ExternalInput")
    w_noise = nc.dram_tensor("w_noise", [E, D], f32, kind="ExternalInput")
    eps = nc.dram_tensor("eps", [T, E], f32, kind="ExternalInput")
    router_out = nc.dram_tensor("router_out", [T, E], f32, kind="ExternalOutput")
    idx_out = nc.dram_tensor("idx_out", [T, K], u32, kind="ExternalOutput")

    # Block-permuted views for the transpose-friendly loads (DMA APs are
    # limited to 3 dims, so one DMA per (group, d-chunk, partition-quarter)).
    # src element (32b + j, 128c + 32a + i) lands at Y_c[32a + j, 32b + i];
    # the DVE 32x32 transpose then yields X_c[32a + i, 32b + j] = mhT.
    mh_v = mh.rearrange(
        "(g b j) (c a i) -> g c a j b i",
        g=G, b=BB, j=32, c=C, a=4, i=32,
    )
    wr_v = w_route.rearrange("(b j) (c a i) -> a b j c i", b=2, j=32, c=C, a=4, i=32)
    wn_v = w_noise.rearrange("(b j) (c a i) -> a b j c i", b=2, j=32, c=C, a=4, i=32)
    eps_v = eps.rearrange("(g q p) e -> g p q e", g=G, q=TAU, p=128)
    rout_v = router_out.rearrange("(g q p) e -> g p q e", g=G, q=TAU, p=128)
    idx_v = idx_out.rearrange("(g q p) k -> g p q k", g=G, q=TAU, p=128)

    with TileContext(nc) as tc:
        ew = nc.gpsimd if gp_elemwise else nc.vector
        with (
            tc.tile_pool(name="const", bufs=1) as const_pool,
            tc.tile_pool(name="ybuf", bufs=3) as y_pool,
            tc.tile_pool(name="xbuf", bufs=4) as x_pool,
            tc.tile_pool(name="acc", bufs=2 * TAU, space="PSUM") as psum_pool,
            tc.tile_pool(name="ep", bufs=2) as ep_pool,
        ):
            # --- W setup: block-load both weight mats, one DVE transpose.
            wall_y = const_pool.tile([128, D], f32)
            wy_v = wall_y.rearrange(
                "(a j) (c b i) -> a b j c i", a=4, j=32, c=C, b=4, i=32
            )
            for a in range(4):
                for b in range(2):
                    nc.sync.dma_start(out=wy_v[a, b], in_=wr_v[a, b])
                    nc.sync.dma_start(out=wy_v[a, 2 + b], in_=wn_v[a, b])
            wallT = const_pool.tile([128, D], f32)
            nc.vector.transpose(wallT[:], wall_y[:])
            # wallT[:, 128c:128c+128][p, e] == Wall[e, 128c + p]

            for g in range(G):
                accs = [
                    psum_pool.tile([128, E2], f32, name=f"acc_g{g}_q{q}", tag="acc")
                    for q in range(TAU)
                ]
                for cg in range(C):
                    y = y_pool.tile([128, TG], f32)
                    y_v = y.rearrange("(a j) (b i) -> a j b i", a=4, j=32, b=BB, i=32)
                    for a in range(4):
                        nc.sync.dma_start(out=y_v[a], in_=mh_v[g, cg, a])
                    x = x_pool.tile([128, TG], f32)
                    nc.vector.transpose(x[:], y[:])
                    for q in range(TAU):
                        nc.tensor.matmul(
                            accs[q],
                            lhsT=x[:, q * 128:(q + 1) * 128],
                            rhs=wallT[:, cg * 128:(cg + 1) * 128],
                            start=(cg == 0),
                            stop=(cg == C - 1),
                        )

                # --- epilogue for this group (tokens on partitions) ---
                L = ep_pool.tile([128, TAU * E2], f32)
                for q in range(TAU):
                    nc.scalar.copy(L[:, q * E2:(q + 1) * E2], accs[q][:, :])
                L3 = L.rearrange("p (q e) -> p q e", e=E2)
                R3 = L3[:, :, 0:E]
                N3 = L3[:, :, E:E2]

                eps_sb = ep_pool.tile([128, TAU * E], f32)
                nc.scalar.dma_start(
                    out=eps_sb.rearrange("p (q e) -> p q e", e=E), in_=eps_v[g]
                )
                eps3 = eps_sb.rearrange("p (q e) -> p q e", e=E)

                en = ep_pool.tile([128, TAU * E], f32)
                en3 = en.rearrange("p (q e) -> p q e", e=E)
                nc.scalar.activation(en3, N3, Exp)
                sp = ep_pool.tile([128, TAU * E], f32)
                sp3 = sp.rearrange("p (q e) -> p q e", e=E)
                nc.scalar.activation(sp3, en3, Ln, bias=1.0)

                nz = ep_pool.tile([128, TAU * E], f32)
                nz3 = nz.rearrange("p (q e) -> p q e", e=E)
                ew.tensor_mul(nz3, eps3, sp3)
                noisy = ep_pool.tile([128, TAU * E], f32)
                noisy3 = noisy.rearrange("p (q e) -> p q e", e=E)
                ew.tensor_add(noisy3, R3, nz3)

                maxv = ep_pool.tile([128, TAU * K], f32)
                maxv3 = maxv.rearrange("p (q k) -> p q k", k=K)
                idxs = ep_pool.tile([128, TAU * K], u32)
                idxs3 = idxs.rearrange("p (q k) -> p q k", k=K)
                for q in range(TAU):
                    nc.vector.max(maxv3[:, q, :], noisy3[:, q, :])
                    nc.vector.max_index(idxs3[:, q, :], maxv3[:, q, :], noisy3[:, q, :])
                nc.scalar.dma_start(out=idx_v[g], in_=idxs3)

                # softmax denominator from the 8 top values
                d8 = ep_pool.tile([128, TAU * K], f32)
                d8_3 = d8.rearrange("p (q k) -> p q k", k=K)
                m1 = maxv3[:, :, 0:1]
                nc.vector.tensor_sub(d8_3, maxv3, m1.broadcast_to([128, TAU, K]))
                e8 = ep_pool.tile([128, TAU * K], f32)
                e8_3 = e8.rearrange("p (q k) -> p q k", k=K)
                nc.scalar.activation(e8_3, d8_3, Exp)
                ssum = ep_pool.tile([128, TAU], f32)
                nc.vector.tensor_reduce(ssum[:, :], e8_3, axis=X, op=add)
                lns = ep_pool.tile([128, TAU], f32)
                nc.scalar.activation(lns[:, :], ssum[:, :], Ln)
                shift = ep_pool.tile([128, TAU], f32)
                nc.vector.tensor_add(
                    shift.rearrange("p q -> p q ()"), m1, lns.rearrange("p q -> p q ()")
                )

                nm = ep_pool.tile([128, TAU * E], f32)
                nm3 = nm.rearrange("p (q e) -> p q e", e=E)
                ew.tensor_sub(
                    nm3,
                    noisy3,
                    shift.rearrange("p q -> p q ()").broadcast_to([128, TAU, E]),
                )
                Ebig = ep_pool.tile([128, TAU * E], f32)
                E3 = Ebig.rearrange("p (q e) -> p q e", e=E)
                nc.scalar.activation(E3, nm3, Exp)
                mask = ep_pool.tile([128, TAU * E], f32)
                mask3 = mask.rearrange("p (q e) -> p q e", e=E)
                thr = maxv3[:, :, K - 1:K]
                ew.tensor_tensor(
                    mask3, noisy3, thr.broadcast_to([128, TAU, E]), op=is_ge
                )
                outp = ep_pool.tile([128, TAU * E], f32)
                outp3 = outp.rearrange("p (q e) -> p q e", e=E)
                ew.tensor_mul(outp3, E3, mask3)
                nc.scalar.dma_start(out=rout_v[g], in_=outp3)

    nc.compile()
    return nc


_NC_CACHE = {}


def _get_nc():
    key = (T_CORE, N_CORES)
    if key not in _NC_CACHE:
        _NC_CACHE[key] = build_nc()
    return _NC_CACHE[key]


def run_sharded(mh_output, W_route, b_route, W_noise, b_noise, noise_eps, top_k,
                **run_kwargs):
    """Shard inputs, run the SPMD kernel, gather. Returns (router, indices, res)."""
    from concourse.bass_utils import run_bass_kernel_spmd

    assert int(top_k) == K, f"kernel hardcodes top_k={K}, got {top_k}"
    mh = np.ascontiguousarray(np.asarray(mh_output, dtype=np.float32)).reshape(T_FULL, D)
    wr = np.ascontiguousarray(np.asarray(W_route, dtype=np.float32))
    wn = np.ascontiguousarray(np.asarray(W_noise, dtype=np.float32))
    ep = np.ascontiguousarray(np.asarray(noise_eps, dtype=np.float32)).reshape(T_FULL, E)
    br = np.asarray(b_route)
    bn = np.asarray(b_noise)
    assert not br.any() and not bn.any(), "kernel assumes zero router biases"

    nc = _get_nc()
    in_maps = []
    for i in range(N_CORES):
        sl = slice(i * T_CORE, (i + 1) * T_CORE)
        in_maps.append(
            {"mh": mh[sl], "w_route": wr, "w_noise": wn, "eps": ep[sl]}
        )
    res = run_bass_kernel_spmd(nc, in_maps, core_ids=list(range(N_CORES)), **run_kwargs)
    router = np.concatenate(
        [res.results[i]["router_out"] for i in range(N_CORES)], axis=0
    ).reshape(B, S, E)
    indices = np.concatenate(
        [res.results[i]["idx_out"].astype(np.int32) for i in range(N_CORES)], axis=0
    ).reshape(B, S, K)
    return router, indices, res


def kernel(mh_output, W_route, b_route, W_noise, b_noise, noise_eps, top_k):
    router, indices, _ = run_sharded(
        mh_output, W_route, b_route, W_noise, b_noise, noise_eps, top_k
    )
    return router, indices


# revision 12
# speedup vs baseline: 1.9632x; 1.9632x over previous
"""NoisyTopkRouter kernel for Trainium2 (8 NeuronCores, data-parallel on tokens).

Math per token t (row of mh):
    logits  = mh @ W_route.T            (+ b_route, asserted zero)
    nlogits = mh @ W_noise.T            (+ b_noise, asserted zero)
    noisy   = logits + eps * softplus(nlogits)
    top-8 of noisy over the 64 experts  -> indices (descending), mask
    router  = softmax(where(mask, noisy, -inf))

Device strategy per core (2048 tokens, D=4096):
  - Both gemms fused into one: Wall = [W_route; W_noise] -> [128, 4096].
  - mh is loaded with big contiguous DMAs ([128 tokens, 4096], 2 MiB each);
    the d-on-partitions operand layout the PE needs is produced on-chip with
    PE transpose-mode (fp32, exact), PSUM -> SBUF copies split ACT/DVE.
  - fp32 matmuls (exact) accumulate [128 tok, 128 e] PSUM tiles over 32
    d-chunks; tokens stay on partitions for the epilogue. Each 128-token
    block accumulates in its own PSUM bank (HW zero-region constraint).
  - softplus = Ln(1 + Exp(x)) on ACT (|x| small, no overflow), top-8 via the
    DVE MAX8 / FIND_INDEX_8 hardware, softmax via exp(noisy - (m + ln(sum))).
"""

import numpy as np

B, S, D, E, K = 4, 4096, 4096, 64, 8
E2 = 2 * E  # stacked route+noise experts
N_CORES = 8
T_FULL = B * S
T_CORE = T_FULL // N_CORES


def build_nc(t_core=T_CORE, n_cores=N_CORES, debug=False, mode="full"):
    import concourse.bacc as bacc
    import concourse.mybir as mybir
    from concourse.masks import make_identity
    from concourse.tile import TileContext

    f32 = mybir.dt.float32
    u32 = mybir.dt.uint32
    Exp = mybir.ActivationFunctionType.Exp
    Ln = mybir.ActivationFunctionType.Ln
    X = mybir.AxisListType.X
    add = mybir.AluOpType.add
    is_ge = mybir.AluOpType.is_ge

    T = t_core
    C = D // 128  # d-chunks (32)
    NT = T // 128  # 128-token tiles
    TAU = min(4, NT)  # token tiles per epilogue group
    G = NT // TAU  # epilogue groups
    CQ = 4  # chunks per psum transpose bank flush
    assert NT % TAU == 0

    nc = bacc.Bacc("TRN2", target_bir_lowering=False, debug=debug, num_devices=n_cores)

    mh = nc.dram_tensor("mh", [T, D], f32, kind="ExternalInput")
    w_route = nc.dram_tensor("w_route", [E, D], f32, kind="ExternalInput")
    w_noise = nc.dram_tensor("w_noise", [E, D], f32, kind="ExternalInput")
    eps = nc.dram_tensor("eps", [T, E], f32, kind="ExternalInput")
    router_out = nc.dram_tensor("router_out", [T, E], f32, kind="ExternalOutput")
    idx_out = nc.dram_tensor("idx_out", [T, K], u32, kind="ExternalOutput")

    mh_v = mh.rearrange("(n p) d -> n p d", p=128)  # contiguous 2 MiB tiles
    # Block-permuted views for the W transpose-friendly load (tiny).
    wr_v = w_route.rearrange("(b j) (c a i) -> a b j c i", b=2, j=32, c=C, a=4, i=32)
    wn_v = w_noise.rearrange("(b j) (c a i) -> a b j c i", b=2, j=32, c=C, a=4, i=32)
    eps_v = eps.rearrange("(g q p) e -> g p q e", g=G, q=TAU, p=128)
    rout_v = router_out.rearrange("(g q p) e -> g p q e", g=G, q=TAU, p=128)
    idx_v = idx_out.rearrange("(g q p) k -> g p q k", g=G, q=TAU, p=128)

    with TileContext(nc) as tc:
        with (
            tc.tile_pool(name="const", bufs=1) as const_pool,
            tc.tile_pool(name="nat", bufs=3) as nat_pool,
            tc.tile_pool(name="xt", bufs=4) as xt_pool,
            tc.tile_pool(name="tp", bufs=2, space="PSUM") as tp_pool,
            tc.tile_pool(name="acc", bufs=6, space="PSUM") as acc_pool,
            tc.tile_pool(name="ep", bufs=2) as ep_pool,
        ):
            # --- identity for PE transpose mode
            ident = const_pool.tile([128, 128], f32)
            make_identity(nc, ident[:, :])

            # --- W setup: block-load both weight mats, one DVE 32x32-block
            # transpose. wallT[:, 128c:128c+128][p, e] == Wall[e, 128c + p]
            wall_y = const_pool.tile([128, D], f32)
            wy_v = wall_y.rearrange(
                "(a j) (c b i) -> a b j c i", a=4, j=32, c=C, b=4, i=32
            )
            for a in range(4):
                for b in range(2):
                    nc.sync.dma_start(out=wy_v[a, b], in_=wr_v[a, b])
                    nc.sync.dma_start(out=wy_v[a, 2 + b], in_=wn_v[a, b])
            wallT = const_pool.tile([128, D], f32)
            nc.vector.transpose(wallT[:], wall_y[:])

            sink = const_pool.tile([128, 16], f32)
            for g in range(G):
                accs = [
                    acc_pool.tile([128, E2], f32, name=f"acc_g{g}_q{q}", tag="acc")
                    for q in range(TAU)
                ]
                for q in range(TAU):
                    nt = g * TAU + q
                    m_nat = nat_pool.tile([128, D], f32)
                    nc.sync.dma_start(out=m_nat[:, :], in_=mh_v[nt])
                    if mode == "dma":
                        nc.vector.tensor_copy(sink[:, 0:4], m_nat[:, 0:4])
                        continue
                    for kb in range(C // CQ):
                        tp = tp_pool.tile([128, CQ * 128], f32)
                        for c4 in range(CQ):
                            cg = kb * CQ + c4
                            nc.tensor.transpose(
                                tp[:, c4 * 128:(c4 + 1) * 128],
                                m_nat[:, cg * 128:(cg + 1) * 128],
                                ident[:, :],
                            )
                        xt = xt_pool.tile([128, CQ * 128], f32)
                        # split PSUM->SBUF copies between ACT and DVE
                        if kb % 2 == 0:
                            nc.scalar.copy(xt[:, :], tp[:, :])
                        else:
                            nc.vector.tensor_copy(xt[:, :], tp[:, :])
                        if mode == "dma+tr":
                            nc.vector.tensor_copy(sink[:, 0:4], xt[:, 0:4])
                            continue
                        for c4 in range(CQ):
                            cg = kb * CQ + c4
                            nc.tensor.matmul(
                                accs[q],
                                lhsT=xt[:, c4 * 128:(c4 + 1) * 128],
                                rhs=wallT[:, cg * 128:(cg + 1) * 128],
                                start=(cg == 0),
                                stop=(cg == C - 1),
                            )

                if mode in ("dma", "dma+tr"):
                    zeros = ep_pool.tile([128, TAU * E], f32)
                    nc.vector.memset(zeros[:, :], 0.0)
                    nc.scalar.dma_start(
                        out=rout_v[g], in_=zeros.rearrange("p (q e) -> p q e", e=E)
                    )
                    zi = ep_pool.tile([128, TAU * K], u32)
                    nc.vector.memset(zi[:, :], 0)
                    nc.scalar.dma_start(
                        out=idx_v[g], in_=zi.rearrange("p (q k) -> p q k", k=K)
                    )
                    continue

                # --- epilogue for this group (tokens on partitions) ---
                L = ep_pool.tile([128, TAU * E2], f32)
                for q in range(TAU):
                    nc.scalar.copy(L[:, q * E2:(q + 1) * E2], accs[q][:, :])
                L3 = L.rearrange("p (q e) -> p q e", e=E2)
                R3 = L3[:, :, 0:E]
                N3 = L3[:, :, E:E2]

                eps_sb = ep_pool.tile([128, TAU * E], f32)
                nc.scalar.dma_start(
                    out=eps_sb.rearrange("p (q e) -> p q e", e=E), in_=eps_v[g]
                )
                eps3 = eps_sb.rearrange("p (q e) -> p q e", e=E)

                en = ep_pool.tile([128, TAU * E], f32)
                en3 = en.rearrange("p (q e) -> p q e", e=E)
                nc.scalar.activation(en3, N3, Exp)
                sp = ep_pool.tile([128, TAU * E], f32)
                sp3 = sp.rearrange("p (q e) -> p q e", e=E)
                nc.scalar.activation(sp3, en3, Ln, bias=1.0)

                nz = ep_pool.tile([128, TAU * E], f32)
                nz3 = nz.rearrange("p (q e) -> p q e", e=E)
                nc.vector.tensor_mul(nz3, eps3, sp3)
                noisy = ep_pool.tile([128, TAU * E], f32)
                noisy3 = noisy.rearrange("p (q e) -> p q e", e=E)
                nc.vector.tensor_add(noisy3, R3, nz3)

                maxv = ep_pool.tile([128, TAU * K], f32)
                maxv3 = maxv.rearrange("p (q k) -> p q k", k=K)
                idxs = ep_pool.tile([128, TAU * K], u32)
                idxs3 = idxs.rearrange("p (q k) -> p q k", k=K)
                for q in range(TAU):
                    nc.vector.max(maxv3[:, q, :], noisy3[:, q, :])
                    nc.vector.max_index(idxs3[:, q, :], maxv3[:, q, :], noisy3[:, q, :])
                nc.scalar.dma_start(out=idx_v[g], in_=idxs3)

                # softmax denominator from the 8 top values
                d8 = ep_pool.tile([128, TAU * K], f32)
                d8_3 = d8.rearrange("p (q k) -> p q k", k=K)
                m1 = maxv3[:, :, 0:1]
                nc.vector.tensor_sub(d8_3, maxv3, m1.broadcast_to([128, TAU, K]))
                e8 = ep_pool.tile([128, TAU * K], f32)
                e8_3 = e8.rearrange("p (q k) -> p q k", k=K)
                nc.scalar.activation(e8_3, d8_3, Exp)
                ssum = ep_pool.tile([128, TAU], f32)
                nc.vector.tensor_reduce(ssum[:, :], e8_3, axis=X, op=add)
                lns = ep_pool.tile([128, TAU], f32)
                nc.scalar.activation(lns[:, :], ssum[:, :], Ln)
                shift = ep_pool.tile([128, TAU], f32)
                nc.vector.tensor_add(
                    shift.rearrange("p q -> p q ()"), m1, lns.rearrange("p q -> p q ()")
                )

                nm = ep_pool.tile([128, TAU * E], f32)
                nm3 = nm.rearrange("p (q e) -> p q e", e=E)
                nc.vector.tensor_sub(
                    nm3,
                    noisy3,
                    shift.rearrange("p q -> p q ()").broadcast_to([128, TAU, E]),
                )
                Ebig = ep_pool.tile([128, TAU * E], f32)
                E3 = Ebig.rearrange("p (q e) -> p q e", e=E)
                nc.scalar.activation(E3, nm3, Exp)
                mask = ep_pool.tile([128, TAU * E], f32)
                mask3 = mask.rearrange("p (q e) -> p q e", e=E)
                thr = maxv3[:, :, K - 1:K]
                nc.vector.tensor_tensor(
                    mask3, noisy3, thr.broadcast_to([128, TAU, E]), op=is_ge
                )
                outp = ep_pool.tile([128, TAU * E], f32)
                outp3 = outp.rearrange("p (q e) -> p q e", e=E)
                nc.vector.tensor_mul(outp3, E3, mask3)
                nc.scalar.dma_start(out=rout_v[g], in_=outp3)

    nc.compile()
    return nc


_NC_CACHE = {}


def _get_nc():
    key = (T_CORE, N_CORES)
    if key not in _NC_CACHE:
        _NC_CACHE[key] = build_nc()
    return _NC_CACHE[key]


def run_sharded(mh_output, W_route, b_route, W_noise, b_noise, noise_eps, top_k,
                **run_kwargs):
    """Shard inputs, run the SPMD kernel, gather. Returns (router, indices, res)."""
    from concourse.bass_utils import run_bass_kernel_spmd

    assert int(top_k) == K, f"kernel hardcodes top_k={K}, got {top_k}"
    mh = np.ascontiguousarray(np.asarray(mh_output, dtype=np.float32)).reshape(T_FULL, D)
    wr = np.ascontiguousarray(np.asarray(W_route, dtype=np.float32))
    wn = np.ascontiguousarray(np.asarray(W_noise, dtype=np.float32))
    ep = np.ascontiguousarray(np.asarray(noise_eps, dtype=np.float32)).reshape(T_FULL, E)
    br = np.asarray(b_route)
    bn = np.asarray(b_noise)
    assert not br.any() and not bn.any(), "kernel assumes zero router biases"

    nc = _get_nc()
    in_maps = []
    for i in range(N_CORES):
        sl = slice(i * T_CORE, (i + 1) * T_CORE)
        in_maps.append(
            {"mh": mh[sl], "w_route": wr, "w_noise": wn, "eps": ep[sl]}
        )
    res = run_bass_kernel_spmd(nc, in_maps, core_ids=list(range(N_CORES)), **run_kwargs)
    router = np.concatenate(
        [res.results[i]["router_out"] for i in range(N_CORES)], axis=0
    ).reshape(B, S, E)
    indices = np.concatenate(
        [res.results[i]["idx_out"].astype(np.int32) for i in range(N_CORES)], axis=0
    ).reshape(B, S, K)
    return router, indices, res


def kernel(mh_output, W_route, b_route, W_noise, b_noise, noise_eps, top_k):
    router, indices, _ = run_sharded(
        mh_output, W_route, b_route, W_noise, b_noise, noise_eps, top_k
    )
    return router, indices


# revision 16
# speedup vs baseline: 4.0923x; 2.0845x over previous
"""NoisyTopkRouter kernel for Trainium2 (8 NeuronCores, data-parallel on tokens).

Math per token t (row of mh):
    logits  = mh @ W_route.T            (+ b_route, asserted zero)
    nlogits = mh @ W_noise.T            (+ b_noise, asserted zero)
    noisy   = logits + eps * softplus(nlogits)
    top-8 of noisy over the 64 experts  -> indices (descending), mask
    router  = softmax(where(mask, noisy, -inf))

Device strategy per core (2048 tokens, D=4096):
  - Both gemms fused into one: Wall = [W_route; W_noise] -> [128, 4096].
  - mh is loaded with big contiguous DMAs ([128 tokens, 4096], 2 MiB each);
    the d-on-partitions operand layout the PE needs is produced on-chip with
    PE transpose-mode (fp32, exact), PSUM -> SBUF copies split ACT/DVE.
  - fp32 matmuls (exact) accumulate [128 tok, 128 e] PSUM tiles over 32
    d-chunks; tokens stay on partitions for the epilogue. Each 128-token
    block accumulates in its own PSUM bank (HW zero-region constraint).
  - softplus = Ln(1 + Exp(x)) on ACT (|x| small, no overflow), top-8 via the
    DVE MAX8 / FIND_INDEX_8 hardware, softmax via exp(noisy - (m + ln(sum))).
"""

import numpy as np

B, S, D, E, K = 4, 4096, 4096, 64, 8
E2 = 2 * E  # stacked route+noise experts
N_CORES = 8
T_FULL = B * S
T_CORE = T_FULL // N_CORES


def build_nc(t_core=T_CORE, n_cores=N_CORES, debug=False, mode="full", repeat=1,
             tr_f32r=False):
    import concourse.bacc as bacc
    import concourse.mybir as mybir
    from concourse.masks import make_identity
    from concourse.tile import TileContext

    f32 = mybir.dt.float32
    f32r = mybir.dt.float32r
    u32 = mybir.dt.uint32
    Exp = mybir.ActivationFunctionType.Exp
    Ln = mybir.ActivationFunctionType.Ln
    X = mybir.AxisListType.X
    add = mybir.AluOpType.add
    is_ge = mybir.AluOpType.is_ge

    T = t_core
    C = D // 128  # d-chunks (32)
    NT = T // 128  # 128-token tiles
    TAU = min(4, NT)  # token tiles per epilogue group
    G = NT // TAU  # epilogue groups
    CQ = 4  # chunks per psum transpose bank flush
    assert NT % TAU == 0

    nc = bacc.Bacc("TRN2", target_bir_lowering=False, debug=debug, num_devices=n_cores)

    mh = nc.dram_tensor("mh", [T, D], f32, kind="ExternalInput")
    w_route = nc.dram_tensor("w_route", [E, D], f32, kind="ExternalInput")
    w_noise = nc.dram_tensor("w_noise", [E, D], f32, kind="ExternalInput")
    eps = nc.dram_tensor("eps", [T, E], f32, kind="ExternalInput")
    router_out = nc.dram_tensor("router_out", [T, E], f32, kind="ExternalOutput")
    idx_out = nc.dram_tensor("idx_out", [T, K], u32, kind="ExternalOutput")

    mh_v = mh.rearrange("(n p) d -> n p d", p=128)  # contiguous 2 MiB tiles
    # Block-permuted views for the W transpose-friendly load (tiny).
    wr_v = w_route.rearrange("(b j) (c a i) -> a b j c i", b=2, j=32, c=C, a=4, i=32)
    wn_v = w_noise.rearrange("(b j) (c a i) -> a b j c i", b=2, j=32, c=C, a=4, i=32)
    eps_v = eps.rearrange("(g q p) e -> g p q e", g=G, q=TAU, p=128)
    rout_v = router_out.rearrange("(g q p) e -> g p q e", g=G, q=TAU, p=128)
    idx_v = idx_out.rearrange("(g q p) k -> g p q k", g=G, q=TAU, p=128)

    with TileContext(nc) as tc:
        with (
            tc.tile_pool(name="const", bufs=1) as const_pool,
            tc.tile_pool(name="nat", bufs=3) as nat_pool,
            tc.tile_pool(name="xt", bufs=4) as xt_pool,
            tc.tile_pool(name="tp", bufs=2, space="PSUM") as tp_pool,
            tc.tile_pool(name="acc", bufs=6, space="PSUM") as acc_pool,
            tc.tile_pool(name="ep", bufs=2) as ep_pool,
        ):
            # --- identity for PE transpose mode
            ident = const_pool.tile([128, 128], f32)
            make_identity(nc, ident[:, :])

            # --- W setup: block-load both weight mats, one DVE 32x32-block
            # transpose. wallT[:, 128c:128c+128][p, e] == Wall[e, 128c + p]
            wall_y = const_pool.tile([128, D], f32)
            wy_v = wall_y.rearrange(
                "(a j) (c b i) -> a b j c i", a=4, j=32, c=C, b=4, i=32
            )
            for a in range(4):
                for b in range(2):
                    nc.sync.dma_start(out=wy_v[a, b], in_=wr_v[a, b])
                    nc.sync.dma_start(out=wy_v[a, 2 + b], in_=wn_v[a, b])
            wallT = const_pool.tile([128, D], f32)
            nc.vector.transpose(wallT[:], wall_y[:])

            sink = const_pool.tile([128, 16], f32)
            for _rep in range(repeat):
              for g in range(G):
                accs = [
                    acc_pool.tile([128, E2], f32, name=f"acc_g{g}_q{q}", tag="acc")
                    for q in range(TAU)
                ]
                for q in range(TAU):
                    nt = g * TAU + q
                    m_nat = nat_pool.tile([128, D], f32)
                    nc.sync.dma_start(out=m_nat[:, :], in_=mh_v[nt])
                    if mode == "dma":
                        nc.vector.tensor_copy(sink[:, 0:4], m_nat[:, 0:4])
                        continue
                    for kb in range(C // CQ):
                        tp = tp_pool.tile([128, CQ * 128], f32)
                        for c4 in range(CQ):
                            cg = kb * CQ + c4
                            tsrc = m_nat[:, cg * 128:(cg + 1) * 128]
                            tid = ident[:, :]
                            tdst = tp[:, c4 * 128:(c4 + 1) * 128]
                            if tr_f32r:
                                tsrc = tsrc.bitcast(f32r)
                                tid = tid.bitcast(f32r)
                                tdst = tdst.bitcast(f32r)
                            nc.tensor.transpose(tdst, tsrc, tid)
                        xt = xt_pool.tile([128, CQ * 128], f32)
                        # split PSUM->SBUF copies between ACT and DVE
                        if kb % 2 == 0:
                            nc.scalar.copy(xt[:, :], tp[:, :])
                        else:
                            nc.vector.tensor_copy(xt[:, :], tp[:, :])
                        if mode == "dma+tr":
                            nc.vector.tensor_copy(sink[:, 0:4], xt[:, 0:4])
                            continue
                        for c4 in range(CQ):
                            cg = kb * CQ + c4
                            nc.tensor.matmul(
                                accs[q],
                                lhsT=xt[:, c4 * 128:(c4 + 1) * 128],
                                rhs=wallT[:, cg * 128:(cg + 1) * 128],
                                start=(cg == 0),
                                stop=(cg == C - 1),
                            )

                if mode in ("dma", "dma+tr"):
                    zeros = ep_pool.tile([128, TAU * E], f32)
                    nc.vector.memset(zeros[:, :], 0.0)
                    nc.scalar.dma_start(
                        out=rout_v[g], in_=zeros.rearrange("p (q e) -> p q e", e=E)
                    )
                    zi = ep_pool.tile([128, TAU * K], u32)
                    nc.vector.memset(zi[:, :], 0)
                    nc.scalar.dma_start(
                        out=idx_v[g], in_=zi.rearrange("p (q k) -> p q k", k=K)
                    )
                    continue

                # --- epilogue for this group (tokens on partitions) ---
                L = ep_pool.tile([128, TAU * E2], f32)
                for q in range(TAU):
                    nc.scalar.copy(L[:, q * E2:(q + 1) * E2], accs[q][:, :])
                L3 = L.rearrange("p (q e) -> p q e", e=E2)
                R3 = L3[:, :, 0:E]
                N3 = L3[:, :, E:E2]

                eps_sb = ep_pool.tile([128, TAU * E], f32)
                nc.scalar.dma_start(
                    out=eps_sb.rearrange("p (q e) -> p q e", e=E), in_=eps_v[g]
                )
                eps3 = eps_sb.rearrange("p (q e) -> p q e", e=E)

                en = ep_pool.tile([128, TAU * E], f32)
                en3 = en.rearrange("p (q e) -> p q e", e=E)
                nc.scalar.activation(en3, N3, Exp)
                sp = ep_pool.tile([128, TAU * E], f32)
                sp3 = sp.rearrange("p (q e) -> p q e", e=E)
                nc.scalar.activation(sp3, en3, Ln, bias=1.0)

                nz = ep_pool.tile([128, TAU * E], f32)
                nz3 = nz.rearrange("p (q e) -> p q e", e=E)
                nc.vector.tensor_mul(nz3, eps3, sp3)
                noisy = ep_pool.tile([128, TAU * E], f32)
                noisy3 = noisy.rearrange("p (q e) -> p q e", e=E)
                nc.vector.tensor_add(noisy3, R3, nz3)

                maxv = ep_pool.tile([128, TAU * K], f32)
                maxv3 = maxv.rearrange("p (q k) -> p q k", k=K)
                idxs = ep_pool.tile([128, TAU * K], u32)
                idxs3 = idxs.rearrange("p (q k) -> p q k", k=K)
                for q in range(TAU):
                    nc.vector.max(maxv3[:, q, :], noisy3[:, q, :])
                    nc.vector.max_index(idxs3[:, q, :], maxv3[:, q, :], noisy3[:, q, :])
                nc.scalar.dma_start(out=idx_v[g], in_=idxs3)

                # softmax denominator from the 8 top values
                d8 = ep_pool.tile([128, TAU * K], f32)
                d8_3 = d8.rearrange("p (q k) -> p q k", k=K)
                m1 = maxv3[:, :, 0:1]
                nc.vector.tensor_sub(d8_3, maxv3, m1.broadcast_to([128, TAU, K]))
                e8 = ep_pool.tile([128, TAU * K], f32)
                e8_3 = e8.rearrange("p (q k) -> p q k", k=K)
                nc.scalar.activation(e8_3, d8_3, Exp)
                ssum = ep_pool.tile([128, TAU], f32)
                nc.vector.tensor_reduce(ssum[:, :], e8_3, axis=X, op=add)
                lns = ep_pool.tile([128, TAU], f32)
                nc.scalar.activation(lns[:, :], ssum[:, :], Ln)
                shift = ep_pool.tile([128, TAU], f32)
                nc.vector.tensor_add(
                    shift.rearrange("p q -> p q ()"), m1, lns.rearrange("p q -> p q ()")
                )

                nm = ep_pool.tile([128, TAU * E], f32)
                nm3 = nm.rearrange("p (q e) -> p q e", e=E)
                nc.vector.tensor_sub(
                    nm3,
                    noisy3,
                    shift.rearrange("p q -> p q ()").broadcast_to([128, TAU, E]),
                )
                Ebig = ep_pool.tile([128, TAU * E], f32)
                E3 = Ebig.rearrange("p (q e) -> p q e", e=E)
                nc.scalar.activation(E3, nm3, Exp)
                mask = ep_pool.tile([128, TAU * E], f32)
                mask3 = mask.rearrange("p (q e) -> p q e", e=E)
                thr = maxv3[:, :, K - 1:K]
                nc.vector.tensor_tensor(
                    mask3, noisy3, thr.broadcast_to([128, TAU, E]), op=is_ge
                )
                outp = ep_pool.tile([128, TAU * E], f32)
                outp3 = outp.rearrange("p (q e) -> p q e", e=E)
                nc.vector.tensor_mul(outp3, E3, mask3)
                nc.scalar.dma_start(out=rout_v[g], in_=outp3)

    nc.compile()
    return nc


_NC_CACHE = {}


def _get_nc():
    key = (T_CORE, N_CORES)
    if key not in _NC_CACHE:
        _NC_CACHE[key] = build_nc()
    return _NC_CACHE[key]


def run_sharded(mh_output, W_route, b_route, W_noise, b_noise, noise_eps, top_k,
                **run_kwargs):
    """Shard inputs, run the SPMD kernel, gather. Returns (router, indices, res)."""
    from concourse.bass_utils import run_bass_kernel_spmd

    assert int(top_k) == K, f"kernel hardcodes top_k={K}, got {top_k}"
    mh = np.ascontiguousarray(np.asarray(mh_output, dtype=np.float32)).reshape(T_FULL, D)
    wr = np.ascontiguousarray(np.asarray(W_route, dtype=np.float32))
    wn = np.ascontiguousarray(np.asarray(W_noise, dtype=np.float32))
    ep = np.ascontiguousarray(np.asarray(noise_eps, dtype=np.float32)).reshape(T_FULL, E)
    br = np.asarray(b_route)
    bn = np.asarray(b_noise)
    assert not br.any() and not bn.any(), "kernel assumes zero router biases"

    nc = _get_nc()
    in_maps = []
    for i in range(N_CORES):
        sl = slice(i * T_CORE, (i + 1) * T_CORE)
        in_maps.append(
            {"mh": mh[sl], "w_route": wr, "w_noise": wn, "eps": ep[sl]}
        )
    res = run_bass_kernel_spmd(nc, in_maps, core_ids=list(range(N_CORES)), **run_kwargs)
    router = np.concatenate(
        [res.results[i]["router_out"] for i in range(N_CORES)], axis=0
    ).reshape(B, S, E)
    indices = np.concatenate(
        [res.results[i]["idx_out"].astype(np.int32) for i in range(N_CORES)], axis=0
    ).reshape(B, S, K)
    return router, indices, res


def kernel(mh_output, W_route, b_route, W_noise, b_noise, noise_eps, top_k):
    router, indices, _ = run_sharded(
        mh_output, W_route, b_route, W_noise, b_noise, noise_eps, top_k
    )
    return router, indices
